# revision 12
# baseline (speedup 1.0000x reference)
# LSTM (embedding -> single-layer LSTM -> linear head) on Trainium2.
#
# Sharding: data-parallel over batch, B=64 -> 2 cores x 32. 32 is the max
# batch per core that lets the 4 LSTM gates be computed as 4 column-tiled
# matmuls filling one PSUM bank completely: partition groups
# {0-31: i, 32-63: f, 64-95: o, 96-127: g} (gate order permuted on host).
#
# Per core, fused pipeline:
#   producer (per 128-token chunk = 4 timesteps x 32 batch): indices DMA ->
#     indirect-DMA embedding gather -> PE transpose -> x_proj GEMM (+bias
#     via ones-row matmul) -> SBUF chunk buffer (double buffered).
#   recurrence (per step): 4 col-tiled matmuls accumulate h @ w_hh.T into
#     the psum gate groups + a diagonal-identity matmul injects x_proj;
#     sigmoid over partitions 0..95 in one ACT op, tanh for g; f/o/g are
#     rebased to partition 0 with identity matmuls (engines require equal
#     operand start partitions); DVE c/h updates; PE-transpose of h back
#     into lhsT layout for the next step.
# Matmuls use float32r (e8m11, 1 cycle/row at N=512); inputs to f32r
# matmuls are rounded on-device by their producer ops (walrus requirement).
import numpy as np

_VOCAB, _EMB, _HID = 50257, 512, 512
_B, _T = 64, 1024
_NCORES = 2
_BLOC = 32


def _build(T, vocab, bloc=32):
    """Build the per-core Bass program. Returns (nc, input_names, out_name)."""
    import concourse.bass as bass
    import concourse.mybir as mybir
    import concourse.tile as tile
    from concourse import bacc
    from concourse.masks import make_identity

    f32 = mybir.dt.float32
    f32r = mybir.dt.float32r
    bf16 = mybir.dt.bfloat16
    i32 = mybir.dt.int32
    SIG = mybir.ActivationFunctionType.Sigmoid
    TANH = mybir.ActivationFunctionType.Tanh
    MUL = mybir.AluOpType.mult
    ADD = mybir.AluOpType.add

    TPC = 128 // bloc            # timesteps per 128-token chunk (4)
    assert T % TPC == 0
    NCH = T // TPC               # number of chunks
    H4 = 4 * _HID

    nc = bacc.Bacc("TRN2", target_bir_lowering=False, debug=False)

    # ---- DRAM I/O (per-core) ----
    emb_d = nc.dram_tensor("emb", [vocab, _EMB], f32, kind="ExternalInput")
    xidx_d = nc.dram_tensor("xidx", [bloc * T, 1], i32, kind="ExternalInput")
    whhT_d = nc.dram_tensor("whhT", [128, 4, H4], bf16, kind="ExternalInput")
    wihT_d = nc.dram_tensor("wihT", [128, 4, H4], f32, kind="ExternalInput")
    bias_d = nc.dram_tensor("bias", [1, H4], f32, kind="ExternalInput")
    woutT_d = nc.dram_tensor("woutT", [128, 4, 2], bf16, kind="ExternalInput")
    bout_d = nc.dram_tensor("bout", [1, 2], bf16, kind="ExternalInput")
    out_d = nc.dram_tensor("out", [bloc, 2], f32, kind="ExternalOutput")

    with tile.TileContext(nc) as tc:
        with (
            tc.tile_pool(name="const", bufs=1) as pc,
            tc.tile_pool(name="stage", bufs=1) as pstg,
            tc.tile_pool(name="state", bufs=1) as ps,
            tc.tile_pool(name="idx", bufs=2) as pidx,
            tc.tile_pool(name="emb", bufs=2) as pe,
            tc.tile_pool(name="eT", bufs=2) as peT,
            tc.tile_pool(name="xb", bufs=2) as pxb,
            tc.tile_pool(name="act", bufs=2) as pact,
            tc.tile_pool(name="hh", bufs=2) as ph,
            tc.tile_pool(name="hT", bufs=2) as phT,
            tc.tile_pool(name="tmp", bufs=2) as ptmp,
            tc.tile_pool(name="ps_g", bufs=2, space="PSUM") as pp_g,
            tc.tile_pool(name="ps_r", bufs=1, space="PSUM") as pp_r,
            tc.tile_pool(name="ps_t", bufs=1, space="PSUM") as pp_t,
            tc.tile_pool(name="ps_e", bufs=1, space="PSUM") as pp_e,
            tc.tile_pool(name="ps_x", bufs=1, space="PSUM") as pp_x,
        ):
            # ---- weights: DMA fp32 staging -> round into f32r tiles ----
            whhT = pc.tile([128, 4, H4], bf16)
            nc.sync.dma_start(whhT[:], whhT_d.ap())
            wihT = pc.tile([128, 4, H4], f32r)
            stg = pstg.tile([128, 4, H4], f32, tag="stg2")
            nc.sync.dma_start(stg[:], wihT_d.ap())
            nc.vector.tensor_copy(wihT[:], stg[:])

            bias_t = pc.tile([1, H4], f32r)
            stg = pstg.tile([1, H4], f32, tag="stg_b")
            nc.sync.dma_start(stg[:], bias_d.ap())
            nc.vector.tensor_copy(bias_t[:], stg[:])
            woutT = pc.tile([128, 4, 2], bf16)
            nc.sync.dma_start(woutT[:], woutT_d.ap())
            bout_t = pc.tile([1, 2], bf16)
            nc.sync.dma_start(bout_t[:], bout_d.ap())

            ident = pc.tile([128, 128], f32)      # for PE transposes
            make_identity(nc, ident[:])
            identr = pc.tile([128, 128], f32r)    # for f32r inject/rebase matmuls
            nc.vector.tensor_copy(identr[:], ident[:])
            identb = pc.tile([128, 128], bf16)    # for bf16 inject matmuls
            nc.vector.tensor_copy(identb[:], ident[:])
            ones_f = pc.tile([1, 128], f32)
            nc.vector.memset(ones_f[:], 1.0)
            ones_r = pc.tile([1, 128], f32r)
            nc.vector.tensor_copy(ones_r[:], ones_f[:])
            zero_b = pc.tile([128, 1], f32)       # explicit ACT bias
            nc.vector.memset(zero_b[:], 0.0)

            # persistent state
            c_sb = ps.tile([bloc, _HID], f32)
            nc.vector.memset(c_sb[:], 0.0)

            # ---- x_proj producer: one 128-token chunk (TPC steps) ----
            def produce_chunk(cb):
                idx_t = pidx.tile([128, 1], i32)
                nc.sync.dma_start(idx_t[:], xidx_d.ap()[cb * 128:(cb + 1) * 128, :])
                e_t = pe.tile([128, _EMB], f32)
                nc.gpsimd.indirect_dma_start(
                    out=e_t[:],
                    out_offset=None,
                    in_=emb_d.ap(),
                    in_offset=bass.IndirectOffsetOnAxis(ap=idx_t[:, :1], axis=0),
                )
                ps_e = pp_e.tile([128, 4, 128], f32, space="PSUM")
                for k in range(4):
                    nc.tensor.transpose(ps_e[:, k, :], e_t[:, k * 128:(k + 1) * 128], ident[:])
                eT = peT.tile([128, 4, 128], f32r)
                nc.vector.tensor_copy(eT[:], ps_e[:])
                xb0 = pxb.tile([128, 512], f32r, tag="xb0")
                xb123 = pxb.tile([128, 3, 512], bf16, tag="xb123")
                for j in range(4):
                    ps_x = pp_x.tile([128, 512], f32, space="PSUM")
                    for k in range(4):
                        nc.tensor.matmul(
                            ps_x[:], eT[:, k, :], wihT[:, k, j * 512:(j + 1) * 512],
                            start=(k == 0), stop=False,
                        )
                    nc.tensor.matmul(
                        ps_x[:], ones_r[0:1, 0:128], bias_t[0:1, j * 512:(j + 1) * 512],
                        start=False, stop=True,
                    )
                    if j == 0:
                        nc.scalar.copy(xb0[:, :], ps_x[:])
                    elif j == 1:
                        nc.scalar.copy(xb123[:, 0, :], ps_x[:])
                    else:
                        nc.vector.tensor_copy(xb123[:, j - 1, :], ps_x[:])
                return (xb0, xb123)

            # ---- one recurrence step ----
            # hT: [128, 4*bloc] f32r (h transposed; K-chunk k at cols
            # k*bloc..(k+1)*bloc), or None at t=0.
            def step(t, xb, hT):
                r = t % TPC
                rp = r * bloc
                xb0, xb123 = xb
                ps_g = pp_g.tile([128, 512], f32, space="PSUM")
                if hT is not None:
                    for k in range(4):
                        for j in range(4):
                            nc.tensor.matmul(
                                ps_g[j * bloc:(j + 1) * bloc, :],
                                hT[:, k * bloc:(k + 1) * bloc],
                                whhT[:, k, j * 512:(j + 1) * 512],
                                start=(k == 0), stop=False,
                                tile_position=(0, j * bloc),
                                skip_group_check=True,
                            )
                nc.tensor.matmul(
                    ps_g[0:bloc, :],
                    identr[rp:rp + bloc, rp:rp + bloc],
                    xb0[rp:rp + bloc, :],
                    start=(hT is None), stop=True,
                    tile_position=(rp, 0),
                    skip_group_check=True,
                )
                for j in range(1, 4):
                    nc.tensor.matmul(
                        ps_g[j * bloc:(j + 1) * bloc, :],
                        identb[rp:rp + bloc, rp:rp + bloc],
                        xb123[rp:rp + bloc, j - 1, :],
                        start=(hT is None), stop=True,
                        tile_position=(rp, j * bloc),
                        skip_group_check=True,
                    )
                act = pact.tile([128, 512], f32r)
                # gates are [i, f, o, g] on partition groups of 32:
                # sigmoid(i, f, o) in one op over partitions 0..95.
                nc.scalar.activation(act[0:96, :], ps_g[0:96, :], SIG,
                                     bias=zero_b[0:96, 0:1])
                nc.scalar.activation(act[96:128, :], ps_g[96:128, :], TANH,
                                     bias=zero_b[96:128, 0:1])
                # rebase f, o, g to partition base 0 (separate psum banks)
                ps_r = pp_r.tile([128, 3, 512], f32, space="PSUM")
                nc.tensor.matmul(ps_r[0:bloc, 0, :], identr[32:64, 32:64],
                                 act[32:64, :], start=True, stop=True,
                                 tile_position=(32, 0))
                nc.tensor.matmul(ps_r[0:bloc, 1, :], identr[64:96, 64:96],
                                 act[64:96, :], start=True, stop=True,
                                 tile_position=(64, 0))
                nc.tensor.matmul(ps_r[0:bloc, 2, :], identr[96:128, 96:128],
                                 act[96:128, :], start=True, stop=True,
                                 tile_position=(96, 0))

                ig = ptmp.tile([bloc, _HID], f32, tag="ig")
                nc.vector.tensor_tensor(ig[:], act[0:bloc, :], ps_r[0:bloc, 2, :], MUL)
                fc = ptmp.tile([bloc, _HID], f32, tag="fc")
                nc.vector.tensor_tensor(fc[:], ps_r[0:bloc, 0, :], c_sb[:], MUL)
                nc.vector.tensor_tensor(c_sb[:], fc[:], ig[:], ADD)
                thc = ptmp.tile([bloc, _HID], f32, tag="thc")
                nc.scalar.activation(thc[:], c_sb[:], TANH, bias=zero_b[0:bloc, 0:1])
                h_sb = ph.tile([bloc, _HID], f32)
                nc.vector.tensor_tensor(h_sb[:], ps_r[0:bloc, 1, :], thc[:], MUL)

                ps_t = pp_t.tile([128, 4 * bloc], f32, space="PSUM")
                for k in range(4):
                    nc.tensor.transpose(
                        ps_t[:, k * bloc:(k + 1) * bloc],
                        h_sb[:, k * 128:(k + 1) * 128],
                        ident[0:bloc, 0:bloc],
                    )
                hT_new = phT.tile([128, 4 * bloc], bf16)
                nc.vector.tensor_copy(hT_new[:], ps_t[:])
                return hT_new

            # ---- main program ----
            xb = produce_chunk(0)
            hT = None
            for cb in range(NCH):
                if cb + 1 < NCH:
                    xb_next = produce_chunk(cb + 1)
                else:
                    xb_next = None
                for rstep in range(TPC):
                    hT = step(cb * TPC + rstep, xb, hT)
                xb = xb_next

            # ---- output head: out = h_last @ w_out.T + b_out ----
            ps_o = pp_t.tile([bloc, 2], f32, space="PSUM", tag="ps_t")
            for k in range(4):
                nc.tensor.matmul(
                    ps_o[:], hT[:, k * bloc:(k + 1) * bloc], woutT[:, k, :],
                    start=(k == 0), stop=False,
                )
            ones_b = pc.tile([1, 128], bf16)
            nc.vector.tensor_copy(ones_b[:], ones_f[:])
            nc.tensor.matmul(
                ps_o[:], ones_b[0:1, 0:bloc], bout_t[0:1, :],
                start=False, stop=True,
            )
            o_sb = pc.tile([bloc, 2], f32)
            nc.vector.tensor_copy(o_sb[:], ps_o[:])
            nc.sync.dma_start(out_d.ap(), o_sb[:])

    nc.compile()
    in_names = ["emb", "xidx", "whhT", "wihT", "bias", "woutT", "bout"]
    return nc, in_names, "out"


def _round_f32r(a):
    """Round fp32 -> e8m11 (f32r storage: top 20 bits), round-to-nearest-even."""
    b = np.asarray(a, np.float32).view(np.uint32)
    lsb = (b >> 12) & 1
    b = (b + 0x7FF + lsb) & np.uint32(0xFFFFF000)
    return b.view(np.float32)


def _prep_host(x, emb, w_ih, w_hh, b_ih, b_hh, w_out, b_out, bloc, ncores):
    """Host-side reshapes: gate permutation [i,f,g,o] -> [i,f,o,g], weight
    transposes into [128, 4, *] K-major tiles, per-core t-major index lists."""
    H = _HID

    def perm_rows(w):
        return np.concatenate([w[0:H], w[H:2 * H], w[3 * H:4 * H], w[2 * H:3 * H]], axis=0)

    w_ih_p = perm_rows(np.asarray(w_ih, np.float32))
    w_hh_p = perm_rows(np.asarray(w_hh, np.float32))
    bias_p = perm_rows((np.asarray(b_ih, np.float32) + np.asarray(b_hh, np.float32))[:, None])[:, 0]

    # wT[p, k, n] = w_p[n, 128k + p]
    def to_kt(w_p):
        return np.ascontiguousarray(w_p.T.reshape(4, 128, w_p.shape[0]).transpose(1, 0, 2))

    import ml_dtypes
    whhT = to_kt(w_hh_p).astype(ml_dtypes.bfloat16)
    wihT = to_kt(w_ih_p)
    woutT = np.ascontiguousarray(
        np.asarray(w_out, np.float32).T.reshape(4, 128, 2).transpose(1, 0, 2)
    ).astype(ml_dtypes.bfloat16)

    emb_c = np.ascontiguousarray(np.asarray(emb, np.float32))
    bias_c = np.ascontiguousarray(bias_p[None, :])
    bout_c = np.ascontiguousarray(np.asarray(b_out, np.float32)[None, :])

    x = np.asarray(x)
    B, T = x.shape
    in_maps = []
    for c in range(ncores):
        xs = x[c * bloc:(c + 1) * bloc, :]          # [bloc, T]
        xidx = np.ascontiguousarray(xs.T.reshape(bloc * T, 1)).astype(np.int32)
        in_maps.append({
            "emb": emb_c,
            "xidx": xidx,
            "whhT": whhT,
            "wihT": wihT,
            "bias": bias_c,
            "woutT": woutT,
            "bout": bout_c.astype(__import__("ml_dtypes").bfloat16),
        })
    return in_maps


_CACHE = {}


def kernel(x, emb, w_ih, w_hh, b_ih, b_hh, w_out, b_out):
    from concourse.bass_utils import run_bass_kernel_spmd

    x = np.asarray(x)
    B, T = x.shape
    ncores = _NCORES
    bloc = B // ncores
    vocab = emb.shape[0]

    key = (T, vocab, bloc)
    if key not in _CACHE:
        _CACHE[key] = _build(T, vocab, bloc)
    nc, in_names, out_name = _CACHE[key]

    in_maps = _prep_host(x, emb, w_ih, w_hh, b_ih, b_hh, w_out, b_out, bloc, ncores)
    res = run_bass_kernel_spmd(nc, in_maps, core_ids=list(range(ncores)))
    out = np.concatenate([r[out_name] for r in res.results], axis=0)  # [B, 2]
    return out


if __name__ == "__main__":
    _build(_T, _VOCAB, _BLOC)
    print("build ok")


# revision 14
# speedup vs baseline: 1.5829x; 1.5829x over previous
# LSTM (embedding -> single-layer LSTM -> linear head) on Trainium2.
#
# Sharding: data-parallel over batch, B=64 -> 2 cores x 32. 32 is the max
# batch per core that lets the 4 LSTM gates be computed as 4 column-tiled
# matmuls filling one PSUM bank completely: partition groups
# {0-31: i, 32-63: f, 64-95: o, 96-127: g} (gate order permuted on host).
#
# Per core, fused pipeline:
#   producer (per 128-token chunk = 4 timesteps x 32 batch): indices DMA ->
#     indirect-DMA embedding gather -> PE transpose -> x_proj GEMM (+bias
#     via ones-row matmul) -> SBUF chunk buffer (double buffered).
#   recurrence (per step): 4 col-tiled matmuls accumulate h @ w_hh.T into
#     the psum gate groups + a diagonal-identity matmul injects x_proj;
#     sigmoid over partitions 0..95 in one ACT op, tanh for g; f/o/g are
#     rebased to partition 0 with identity matmuls (engines require equal
#     operand start partitions); DVE c/h updates; PE-transpose of h back
#     into lhsT layout for the next step.
# Matmuls use float32r (e8m11, 1 cycle/row at N=512); inputs to f32r
# matmuls are rounded on-device by their producer ops (walrus requirement).
import numpy as np

_VOCAB, _EMB, _HID = 50257, 512, 512
_B, _T = 64, 1024
_NCORES = 2
_BLOC = 32


def _build(T, vocab, bloc=32):
    """Build the per-core Bass program. Returns (nc, input_names, out_name)."""
    import concourse.bass as bass
    import concourse.mybir as mybir
    import concourse.tile as tile
    from concourse import bacc
    from concourse.masks import make_identity

    f32 = mybir.dt.float32
    f32r = mybir.dt.float32r
    bf16 = mybir.dt.bfloat16
    i32 = mybir.dt.int32
    SIG = mybir.ActivationFunctionType.Sigmoid
    TANH = mybir.ActivationFunctionType.Tanh
    MUL = mybir.AluOpType.mult
    ADD = mybir.AluOpType.add

    TPC = 128 // bloc            # timesteps per 128-token chunk (4)
    assert T % TPC == 0
    NCH = T // TPC               # number of chunks
    H4 = 4 * _HID

    nc = bacc.Bacc("TRN2", target_bir_lowering=False, debug=False)

    # ---- DRAM I/O (per-core) ----
    emb_d = nc.dram_tensor("emb", [vocab, _EMB], f32, kind="ExternalInput")
    xidx_d = nc.dram_tensor("xidx", [bloc * T, 1], i32, kind="ExternalInput")
    whhT_d = nc.dram_tensor("whhT", [128, 4, H4], bf16, kind="ExternalInput")
    wihT_d = nc.dram_tensor("wihT", [128, 4, H4], f32, kind="ExternalInput")
    bias_d = nc.dram_tensor("bias", [1, H4], f32, kind="ExternalInput")
    woutT_d = nc.dram_tensor("woutT", [128, 4, 2], bf16, kind="ExternalInput")
    bout_d = nc.dram_tensor("bout", [1, 2], bf16, kind="ExternalInput")
    out_d = nc.dram_tensor("out", [bloc, 2], f32, kind="ExternalOutput")

    with tile.TileContext(nc) as tc:
        with (
            tc.tile_pool(name="const", bufs=1) as pc,
            tc.tile_pool(name="stage", bufs=1) as pstg,
            tc.tile_pool(name="state", bufs=1) as ps,
            tc.tile_pool(name="idx", bufs=2) as pidx,
            tc.tile_pool(name="emb", bufs=2) as pe,
            tc.tile_pool(name="eT", bufs=2) as peT,
            tc.tile_pool(name="xb", bufs=2) as pxb,
            tc.tile_pool(name="act", bufs=2) as pact,
            tc.tile_pool(name="hh", bufs=2) as ph,
            tc.tile_pool(name="hT", bufs=2) as phT,
            tc.tile_pool(name="tmp", bufs=2) as ptmp,
            tc.tile_pool(name="ps_g", bufs=2, space="PSUM") as pp_g,
            tc.tile_pool(name="ps_r", bufs=1, space="PSUM") as pp_r,
            tc.tile_pool(name="ps_t", bufs=2, space="PSUM") as pp_t,
            tc.tile_pool(name="ps_x", bufs=1, space="PSUM") as pp_x,
        ):
            # ---- weights: DMA fp32 staging -> round into f32r tiles ----
            whhT = pc.tile([128, 4, H4], bf16)
            nc.sync.dma_start(whhT[:], whhT_d.ap())
            wihT = pc.tile([128, 4, H4], f32r)
            stg = pstg.tile([128, 4, H4], f32, tag="stg2")
            nc.sync.dma_start(stg[:], wihT_d.ap())
            nc.vector.tensor_copy(wihT[:], stg[:])

            bias_t = pc.tile([1, H4], f32r)
            stg = pstg.tile([1, H4], f32, tag="stg_b")
            nc.sync.dma_start(stg[:], bias_d.ap())
            nc.vector.tensor_copy(bias_t[:], stg[:])
            woutT = pc.tile([128, 4, 2], bf16)
            nc.sync.dma_start(woutT[:], woutT_d.ap())
            bout_t = pc.tile([1, 2], bf16)
            nc.sync.dma_start(bout_t[:], bout_d.ap())

            ident = pc.tile([128, 128], f32)      # for PE transposes
            make_identity(nc, ident[:])
            identr = pc.tile([128, 128], f32r)    # for f32r inject/rebase matmuls
            nc.vector.tensor_copy(identr[:], ident[:])
            identb = pc.tile([128, 128], bf16)    # for bf16 inject matmuls
            nc.vector.tensor_copy(identb[:], ident[:])
            ones_f = pc.tile([1, 128], f32)
            nc.vector.memset(ones_f[:], 1.0)
            ones_r = pc.tile([1, 128], f32r)
            nc.vector.tensor_copy(ones_r[:], ones_f[:])
            zero_b = pc.tile([128, 1], f32)       # explicit ACT bias
            nc.vector.memset(zero_b[:], 0.0)

            # persistent state
            c_sb = ps.tile([bloc, _HID], f32)
            nc.vector.memset(c_sb[:], 0.0)

            # ---- x_proj producer: one 128-token chunk (TPC steps) ----
            def produce_chunk(cb):
                idx_t = pidx.tile([128, 1], i32)
                nc.sync.dma_start(idx_t[:], xidx_d.ap()[cb * 128:(cb + 1) * 128, :])
                e_t = pe.tile([128, _EMB], f32)
                nc.gpsimd.indirect_dma_start(
                    out=e_t[:],
                    out_offset=None,
                    in_=emb_d.ap(),
                    in_offset=bass.IndirectOffsetOnAxis(ap=idx_t[:, :1], axis=0),
                )
                ps_e = pp_x.tile([128, 4, 128], f32, space="PSUM", tag="prod")
                for k in range(4):
                    nc.tensor.transpose(ps_e[:, k, :], e_t[:, k * 128:(k + 1) * 128], ident[:])
                eT = peT.tile([128, 4, 128], f32r)
                nc.scalar.copy(eT[:], ps_e[:])
                xb0 = pxb.tile([128, 512], f32r, tag="xb0")
                xb123 = pxb.tile([128, 3, 512], bf16, tag="xb123")
                for j in range(4):
                    ps_x = pp_x.tile([128, 512], f32, space="PSUM", tag="prod")
                    for k in range(4):
                        nc.tensor.matmul(
                            ps_x[:], eT[:, k, :], wihT[:, k, j * 512:(j + 1) * 512],
                            start=(k == 0), stop=False,
                        )
                    nc.tensor.matmul(
                        ps_x[:], ones_r[0:1, 0:128], bias_t[0:1, j * 512:(j + 1) * 512],
                        start=False, stop=True,
                    )
                    if j == 0:
                        nc.scalar.copy(xb0[:, :], ps_x[:])
                    elif j < 3:
                        nc.scalar.copy(xb123[:, j - 1, :], ps_x[:])
                    else:
                        nc.vector.tensor_copy(xb123[:, j - 1, :], ps_x[:])
                return (xb0, xb123)

            # ---- one recurrence step ----
            # hT: [128, 4*bloc] f32r (h transposed; K-chunk k at cols
            # k*bloc..(k+1)*bloc), or None at t=0.
            def step(t, xb, hT):
                r = t % TPC
                rp = r * bloc
                xb0, xb123 = xb
                ps_g = pp_g.tile([128, 512], f32, space="PSUM")
                last = hT is None
                nc.tensor.matmul(
                    ps_g[0:bloc, :],
                    identr[rp:rp + bloc, rp:rp + bloc],
                    xb0[rp:rp + bloc, :],
                    start=True, stop=last,
                    tile_position=(rp, 0),
                    skip_group_check=True,
                )
                for j in range(1, 4):
                    nc.tensor.matmul(
                        ps_g[j * bloc:(j + 1) * bloc, :],
                        identb[rp:rp + bloc, rp:rp + bloc],
                        xb123[rp:rp + bloc, j - 1, :],
                        start=True, stop=last,
                        tile_position=(rp, j * bloc),
                        skip_group_check=True,
                    )
                if hT is not None:
                    for k in range(4):
                        for j in range(4):
                            nc.tensor.matmul(
                                ps_g[j * bloc:(j + 1) * bloc, :],
                                hT[:, k * bloc:(k + 1) * bloc],
                                whhT[:, k, j * 512:(j + 1) * 512],
                                start=False, stop=(k == 3),
                                tile_position=(0, j * bloc),
                                skip_group_check=True,
                            )
                act = pact.tile([128, 512], f32r)
                # gates are [i, f, o, g] on partition groups of 32:
                # sigmoid(i, f, o) in one op over partitions 0..95.
                nc.scalar.activation(act[0:96, :], ps_g[0:96, :], SIG,
                                     bias=zero_b[0:96, 0:1])
                nc.scalar.activation(act[96:128, :], ps_g[96:128, :], TANH,
                                     bias=zero_b[96:128, 0:1])
                # rebase f, o, g to partition base 0 (separate psum banks)
                ps_r = pp_r.tile([128, 3, 512], f32, space="PSUM")
                nc.tensor.matmul(ps_r[0:bloc, 0, :], identr[32:64, 32:64],
                                 act[32:64, :], start=True, stop=True,
                                 tile_position=(32, 0))
                nc.tensor.matmul(ps_r[0:bloc, 1, :], identr[64:96, 64:96],
                                 act[64:96, :], start=True, stop=True,
                                 tile_position=(64, 0))
                nc.tensor.matmul(ps_r[0:bloc, 2, :], identr[96:128, 96:128],
                                 act[96:128, :], start=True, stop=True,
                                 tile_position=(96, 0))

                ig = ptmp.tile([bloc, _HID], f32, tag="ig")
                nc.vector.tensor_tensor(ig[:], act[0:bloc, :], ps_r[0:bloc, 2, :], MUL)
                fc = ptmp.tile([bloc, _HID], f32, tag="fc")
                nc.vector.tensor_tensor(fc[:], ps_r[0:bloc, 0, :], c_sb[:], MUL)
                nc.vector.tensor_tensor(c_sb[:], fc[:], ig[:], ADD)
                thc = ptmp.tile([bloc, _HID], f32, tag="thc")
                nc.scalar.activation(thc[:], c_sb[:], TANH, bias=zero_b[0:bloc, 0:1])
                h_sb = ph.tile([bloc, _HID], f32)
                nc.vector.tensor_tensor(h_sb[:], ps_r[0:bloc, 1, :], thc[:], MUL)

                ps_t = pp_t.tile([128, 4 * bloc], f32, space="PSUM")
                for k in range(4):
                    nc.tensor.transpose(
                        ps_t[:, k * bloc:(k + 1) * bloc],
                        h_sb[:, k * 128:(k + 1) * 128],
                        ident[0:bloc, 0:bloc],
                    )
                hT_new = phT.tile([128, 4 * bloc], bf16)
                nc.vector.tensor_copy(hT_new[:], ps_t[:])
                return hT_new

            # ---- main program ----
            xb = produce_chunk(0)
            hT = None
            for cb in range(NCH):
                if cb + 1 < NCH:
                    xb_next = produce_chunk(cb + 1)
                else:
                    xb_next = None
                for rstep in range(TPC):
                    hT = step(cb * TPC + rstep, xb, hT)
                xb = xb_next

            # ---- output head: out = h_last @ w_out.T + b_out ----
            ps_o = pp_t.tile([bloc, 2], f32, space="PSUM", tag="ps_t")
            for k in range(4):
                nc.tensor.matmul(
                    ps_o[:], hT[:, k * bloc:(k + 1) * bloc], woutT[:, k, :],
                    start=(k == 0), stop=False,
                )
            ones_b = pc.tile([1, 128], bf16)
            nc.vector.tensor_copy(ones_b[:], ones_f[:])
            nc.tensor.matmul(
                ps_o[:], ones_b[0:1, 0:bloc], bout_t[0:1, :],
                start=False, stop=True,
            )
            o_sb = pc.tile([bloc, 2], f32)
            nc.vector.tensor_copy(o_sb[:], ps_o[:])
            nc.sync.dma_start(out_d.ap(), o_sb[:])

    nc.compile()
    in_names = ["emb", "xidx", "whhT", "wihT", "bias", "woutT", "bout"]
    return nc, in_names, "out"


def _round_f32r(a):
    """Round fp32 -> e8m11 (f32r storage: top 20 bits), round-to-nearest-even."""
    b = np.asarray(a, np.float32).view(np.uint32)
    lsb = (b >> 12) & 1
    b = (b + 0x7FF + lsb) & np.uint32(0xFFFFF000)
    return b.view(np.float32)


def _prep_host(x, emb, w_ih, w_hh, b_ih, b_hh, w_out, b_out, bloc, ncores):
    """Host-side reshapes: gate permutation [i,f,g,o] -> [i,f,o,g], weight
    transposes into [128, 4, *] K-major tiles, per-core t-major index lists."""
    H = _HID

    def perm_rows(w):
        return np.concatenate([w[0:H], w[H:2 * H], w[3 * H:4 * H], w[2 * H:3 * H]], axis=0)

    w_ih_p = perm_rows(np.asarray(w_ih, np.float32))
    w_hh_p = perm_rows(np.asarray(w_hh, np.float32))
    bias_p = perm_rows((np.asarray(b_ih, np.float32) + np.asarray(b_hh, np.float32))[:, None])[:, 0]

    # wT[p, k, n] = w_p[n, 128k + p]
    def to_kt(w_p):
        return np.ascontiguousarray(w_p.T.reshape(4, 128, w_p.shape[0]).transpose(1, 0, 2))

    import ml_dtypes
    whhT = to_kt(w_hh_p).astype(ml_dtypes.bfloat16)
    wihT = to_kt(w_ih_p)
    woutT = np.ascontiguousarray(
        np.asarray(w_out, np.float32).T.reshape(4, 128, 2).transpose(1, 0, 2)
    ).astype(ml_dtypes.bfloat16)

    emb_c = np.ascontiguousarray(np.asarray(emb, np.float32))
    bias_c = np.ascontiguousarray(bias_p[None, :])
    bout_c = np.ascontiguousarray(np.asarray(b_out, np.float32)[None, :])

    x = np.asarray(x)
    B, T = x.shape
    in_maps = []
    for c in range(ncores):
        xs = x[c * bloc:(c + 1) * bloc, :]          # [bloc, T]
        xidx = np.ascontiguousarray(xs.T.reshape(bloc * T, 1)).astype(np.int32)
        in_maps.append({
            "emb": emb_c,
            "xidx": xidx,
            "whhT": whhT,
            "wihT": wihT,
            "bias": bias_c,
            "woutT": woutT,
            "bout": bout_c.astype(__import__("ml_dtypes").bfloat16),
        })
    return in_maps


_CACHE = {}


def kernel(x, emb, w_ih, w_hh, b_ih, b_hh, w_out, b_out):
    from concourse.bass_utils import run_bass_kernel_spmd

    x = np.asarray(x)
    B, T = x.shape
    ncores = _NCORES
    bloc = B // ncores
    vocab = emb.shape[0]

    key = (T, vocab, bloc)
    if key not in _CACHE:
        _CACHE[key] = _build(T, vocab, bloc)
    nc, in_names, out_name = _CACHE[key]

    in_maps = _prep_host(x, emb, w_ih, w_hh, b_ih, b_hh, w_out, b_out, bloc, ncores)
    res = run_bass_kernel_spmd(nc, in_maps, core_ids=list(range(ncores)))
    out = np.concatenate([r[out_name] for r in res.results], axis=0)  # [B, 2]
    return out


if __name__ == "__main__":
    _build(_T, _VOCAB, _BLOC)
    print("build ok")
